# revision 80
# baseline (speedup 1.0000x reference)
"""Multi-head causal attention (B=4, T=2048, C=1024, H=16, DH=64) on 8 trn2
NeuronCores.

Sharding: core = (batch, head-half): core 2*b+g computes heads g*8..g*8+8 of
batch b, including the partial output projection with the matching 512 rows
of Wp (tensor-parallel). Host-side unshard sums the two partials per batch
and adds bp.

v6. Measured hardware facts that shape this kernel:
  - A 512-wide matmul costs ~260ns regardless of dtype (bf16 = fp8 = fp8
    DoubleRow); DoubleRow gives NO speedup on this HW and low-K matmuls
    run SLOWER (activity-throttled clock), so everything is plain bf16
    with full-128 contractions (zero-padded q for the 64-dim QK).
  - The PE clock halves unless the engine stays continuously busy, so
    projection work is software-pipelined between attention chunks
    (deadline pacing for the next block's Q/K/V, ns-accounted pump for
    finished-tj output projections) to keep the PE queue gap-free.
  - Every DMA costs ~625ns on the shared HWDGE engine: inputs arrive as
    a handful of merged partition-major tensors, q/k staging uses only
    partition-preserving DVE copies into resident zero-padded tiles (no
    DMAs at all), and the only per-unit DMAs are the softmax-L bounce
    and the head-1 output row relocation.
  - exp() runs unmasked on ACT straight from PSUM (sole ACT user, the
    ~120us drumbeat); the causal 0/1 tril multiply on P and the
    normalize multiplies run on GPSIMD; PSUM evacuations on DVE.
"""
import math

import numpy as np
import ml_dtypes

import concourse.mybir as mybir
import concourse.tile as tile
from concourse import bacc, bass_utils

F32 = mybir.dt.float32
BF16 = mybir.dt.bfloat16

B, T, C, H, DH = 4, 2048, 1024, 16, 64
HG = H // 2          # heads per core (8)
CC = C // 128        # contraction chunks (8)
HP = HG // 2         # head pairs (4)
TJ = 512             # query chunk width
NTJ = T // TJ        # 4
NSI = T // 128       # 16 key chunks
SCALE = 1.0 / 32.0   # 1/sqrt(C)

NP_BF16 = ml_dtypes.bfloat16

PE_NS = 1.0 / 2.4    # full-clock PE ns/cycle
ACT_NS = 1.0 / 1.2   # ACT ns/elem-row
MM_OH = 47.0         # measured per-matmul overhead ns

TRACE = False
_NC_CACHE = {}


def _build():
    nc = bacc.Bacc(trn_type="TRN2", target_bir_lowering=False, debug=False)

    # partition-major merged input layouts:
    #   xin [128, CC, T]    (p, c, t) = x^T[128c + p, t]
    #   wqk [128, CC, 2*512] (p, c, m) = [Wq | Wk][128c + p, m]
    #   wv  [128, CC, 512]  (p, c, m) = Wv[128c + p, m]
    #   wpm [128, HP, C]    (p, hp, e) = Wp[128hp + p, e]
    xin = nc.dram_tensor("xin", [128, CC, T], BF16, kind="ExternalInput")
    wqk = nc.dram_tensor("wqk", [128, CC, 2 * HG * DH], BF16,
                         kind="ExternalInput")
    wv = nc.dram_tensor("wv", [128, CC, HG * DH], BF16, kind="ExternalInput")
    wpm = nc.dram_tensor("wpm", [128, HP, C], BF16, kind="ExternalInput")
    tril01 = nc.dram_tensor("tril01", [128, 128], BF16, kind="ExternalInput")
    ones8 = nc.dram_tensor("ones8", [128, HG], BF16, kind="ExternalInput")
    zrow = nc.dram_tensor("zrow", [1, HP * TJ], BF16, kind="ExternalInput")
    out = nc.dram_tensor("out", [T, C], BF16, kind="ExternalOutput")

    l_dram = nc.dram_tensor("l_scratch", [HP * NTJ, 2, TJ], F32)

    with tile.TileContext(nc) as tc:
        with (
            tc.tile_pool(name="persist", bufs=1) as persist,

            tc.tile_pool(name="ppool", bufs=7) as ppool,
            tc.tile_pool(name="npool", bufs=2) as npool,
            tc.tile_pool(name="outpool", bufs=3) as outpool,
        ):
            tril_sb = persist.tile([128, 128], BF16)
            nc.sync.dma_start(out=tril_sb, in_=tril01.ap())
            ones_sb = persist.tile([128, HG], BF16)
            nc.sync.dma_start(out=ones_sb, in_=ones8.ap())
            wp_sb = persist.tile([128, HP, C], BF16)

            # k resident [128(= head-pair d rows), hp, T]; q per-tn in the
            # zero-padded per-head form: q0 rows 64:128 = 0, q1 rows 0:64 = 0
            # (zeroed once per rotation buffer; staging copies only touch
            # the live half so the zeros persist).
            k_sb = persist.tile([128, HP, T], BF16)
            v_sb = [persist.tile([128, HG, DH + 1], BF16, name=f"v_{si}")
                    for si in range(NSI)]
            o_sb = [persist.tile([128, T], BF16, name=f"o_{hp}")
                    for hp in range(HP)]
            q_tiles = {}
            for tn in range(NTJ):
                q0 = persist.tile([128, HP, TJ], BF16, name=f"q0_{tn}")
                q1 = persist.tile([128, HP, TJ], BF16, name=f"q1_{tn}")
                q_tiles[tn] = (q0, q1)

            def q_zero_init(tn):
                q0, q1 = q_tiles[tn]
                nc.sync.dma_start(
                    out=q0[64:128, :, :].rearrange("p hp t -> p (hp t)"),
                    in_=zrow.ap().to_broadcast((64, HP * TJ)))
                nc.sync.dma_start(
                    out=q1[0:64, :, :].rearrange("p hp t -> p (hp t)"),
                    in_=zrow.ap().to_broadcast((64, HP * TJ)))

            with (
                tc.tile_pool(name="xw", bufs=1) as xw,
                tc.tile_pool(name="aps", bufs=2, space="PSUM") as aps,
            ):
                wqk_sb = xw.tile([128, CC, 2 * HG * DH], BF16)
                x_sb = xw.tile([128, CC, T], BF16)
                wv_sb = xw.tile([128, CC, HG * DH], BF16)

                # priority order: exactly what tn0's first projection group
                # needs (x quarter 0 then per-mg wqk slices in the mg-major
                # consumption order), then the rest of x and the slow
                # non-critical loads (zero pads, wv, wp).
                nc.sync.dma_start(out=x_sb[:, :, 0:TJ],
                                  in_=xin.ap()[:, :, 0:TJ])
                for mg in range(HP):
                    for base in (mg * 128, 512 + mg * 128):
                        msl = slice(base, base + 128)
                        nc.sync.dma_start(out=wqk_sb[:, :, msl],
                                          in_=wqk.ap()[:, :, msl])
                q_zero_init(0)
                for quarter in range(1, 4):
                    qsl = slice(quarter * TJ, (quarter + 1) * TJ)
                    nc.sync.dma_start(out=x_sb[:, :, qsl],
                                      in_=xin.ap()[:, :, qsl])
                for tn in range(1, NTJ):
                    q_zero_init(tn)

                # ---- per-tn projection streams (generators of sub-items) ----
                def qk_stream(tn):
                    """Q and K projections for block tn (plain bf16, 128
                    contraction chunks), staged by partition-preserving DVE
                    copies into the zero-padded resident tiles."""
                    tsl = slice(tn * TJ, (tn + 1) * TJ)
                    q0, q1 = q_tiles[tn]
                    # mg-major so attention unit hp=mg can start after 1/4
                    # of the projections (it needs only its own Q+K group).
                    for mg in range(HP):
                        for which in ("q", "k"):
                            base = (0 if which == "q" else 512) + mg * 128
                            msl = slice(base, base + 128)
                            pp = aps.tile([128, TJ], F32, name="pp",
                                          tag="pp", bufs=2)
                            for c in range(CC):
                                yield ("mm", TJ * PE_NS + MM_OH,
                                       lambda c=c, pp=pp, msl=msl:
                                       nc.tensor.matmul(
                                           pp, wqk_sb[:, c, msl],
                                           x_sb[:, c, tsl],
                                           start=(c == 0),
                                           stop=(c == CC - 1)))
                            def fin(pp=pp, which=which, mg=mg, q0=q0, q1=q1):
                                if which == "k":
                                    nc.vector.tensor_copy(
                                        k_sb[:, mg, tsl], pp)
                                else:
                                    nc.vector.tensor_copy(
                                        q0[0:64, mg, :], pp[0:64, :])
                                    nc.vector.tensor_copy(
                                        q1[64:128, mg, :], pp[64:128, :])
                            yield ("fin", 0.0, fin)

                def v_stream(tn):
                    for si in range(4 * tn, 4 * tn + 4):
                        ssl = slice(si * 128, (si + 1) * 128)
                        vp = aps.tile([128, HG * DH], F32, name="pp",
                                      tag="pp", bufs=2)
                        for c in range(CC):
                            yield ("mm", HG * DH * PE_NS + MM_OH,
                                   lambda c=c, vp=vp, ssl=ssl:
                                   nc.tensor.matmul(
                                       vp, x_sb[:, c, ssl], wv_sb[:, c, :],
                                       start=(c == 0), stop=(c == CC - 1)))
                        def vfin(vp=vp, si=si):
                            nc.vector.tensor_copy(
                                v_sb[si][:, :, 0:DH],
                                vp.rearrange("p (h d) -> p h d", h=HG))
                            nc.vector.tensor_copy(
                                out=v_sb[si][:, :, DH:DH + 1],
                                in_=ones_sb[:, :, None])
                        yield ("fin", 0.0, vfin)

                def proj_stream(ti):
                    tsl = slice(ti * 128, (ti + 1) * 128)
                    ob = outpool.tile([128, 2, TJ], BF16, name="ob")
                    for en in range(C // TJ):
                        esl = slice(en * TJ, (en + 1) * TJ)
                        op = aps.tile([128, TJ], F32, name="pp", tag="pp",
                                      bufs=2)
                        for hp in range(HP):
                            yield ("mm", TJ * PE_NS + MM_OH,
                                   lambda hp=hp, op=op, tsl=tsl, esl=esl:
                                   nc.tensor.matmul(
                                       op, o_sb[hp][:, tsl],
                                       wp_sb[:, hp, esl],
                                       start=(hp == 0), stop=(hp == HP - 1)))
                        yield ("fin", 0.0,
                               lambda op=op, ob=ob, en=en:
                               nc.vector.tensor_copy(ob[:, en, :], op))
                    yield ("fin", 0.0,
                           lambda ob=ob, tsl=tsl: nc.sync.dma_start(
                               out=out.ap()[tsl, :], in_=ob))

                # tj3's projection is split: hp0..2 partials run during the
                # last attention unit; only the hp3 matmul + add + out DMA
                # remain after the final normalize (shortens the tail).
                partials = {}

                def proj_partial_stream(ti):
                    tsl = slice(ti * 128, (ti + 1) * 128)
                    part = outpool.tile([128, 2, TJ], BF16, name="part",
                                        bufs=4)
                    partials[ti] = part
                    for en in range(C // TJ):
                        esl = slice(en * TJ, (en + 1) * TJ)
                        op = aps.tile([128, TJ], F32, name="pp", tag="pp",
                                      bufs=2)
                        for hp in range(HP - 1):
                            yield ("mm", TJ * PE_NS + MM_OH,
                                   lambda hp=hp, op=op, tsl=tsl, esl=esl:
                                   nc.tensor.matmul(
                                       op, o_sb[hp][:, tsl],
                                       wp_sb[:, hp, esl],
                                       start=(hp == 0), stop=(hp == HP - 2)))
                        yield ("fin", 0.0,
                               lambda op=op, part=part, en=en:
                               nc.vector.tensor_copy(part[:, en, :], op))

                def proj_final(ti):
                    tsl = slice(ti * 128, (ti + 1) * 128)
                    part = partials[ti]
                    ob = outpool.tile([128, 2, TJ], BF16, name="ob")
                    for en in range(C // TJ):
                        esl = slice(en * TJ, (en + 1) * TJ)
                        op = aps.tile([128, TJ], F32, name="pp", tag="pp",
                                      bufs=2)
                        nc.tensor.matmul(
                            op, o_sb[HP - 1][:, tsl], wp_sb[:, HP - 1, esl],
                            start=True, stop=True)
                        nc.vector.tensor_tensor(
                            out=ob[:, en, :], in0=op, in1=part[:, en, :],
                            op=mybir.AluOpType.add)
                    nc.sync.dma_start(out=out.ap()[tsl, :], in_=ob)

                # ---- phase 1: tn=0 Q/K emitted plainly; V + wv stream in
                # via tj0's deadline so they don't block attention start ----
                for kind, cost, fn in qk_stream(0):
                    fn()
                nc.sync.dma_start(out=wv_sb, in_=wv.ap())
                nc.sync.dma_start(out=wp_sb, in_=wpm.ap())

                # ---- pacing machinery ----
                state = {"pe": 0.0, "act": 0.0}
                deadline = {"items": [], "done": 0, "total": 0}
                pump_q = []
                pump_cur = {"g": None}

                def set_deadline(gen):
                    items = list(gen)
                    deadline["items"] = items
                    deadline["done"] = 0
                    deadline["total"] = len(items)

                def deadline_step(frac, min_items=0):
                    need = max(math.ceil(deadline["total"] * min(frac, 1.0)),
                               min(min_items, deadline["total"]))
                    while deadline["done"] < need:
                        kind, cost, fn = deadline["items"][deadline["done"]]
                        fn()
                        state["pe"] += cost
                        deadline["done"] += 1

                def pump():
                    while state["pe"] < state["act"]:
                        if pump_cur["g"] is None:
                            if not pump_q:
                                return
                            pump_cur["g"] = pump_q.pop(0)
                        try:
                            kind, cost, fn = next(pump_cur["g"])
                            fn()
                            state["pe"] += cost
                        except StopIteration:
                            pump_cur["g"] = None

                def drain_pump():
                    state["act"] = float("inf")
                    pump()
                    state["act"] = state["pe"]

                # ---- phase 2: attention ----
                def attn_unit(hp, tj, chunk_base, chunks_total):
                    q0, q1 = q_tiles[tj]
                    o_ps0 = aps.tile([DH + 1, TJ], F32, name="o_ps0", bufs=1)
                    o_ps1 = aps.tile([DH + 1, TJ], F32, name="o_ps1", bufs=1)
                    nsi = 4 * tj + 4
                    prev = None
                    for si in range(nsi):
                        r = si - 4 * tj
                        toff = 0 if r < 0 else 128 * r
                        ssl = slice(si * 128, (si + 1) * 128)
                        s_ps = aps.tile([128, 2, TJ], F32, name="s_ps",
                                        tag="s_ps", bufs=2)
                        nc.tensor.matmul(
                            s_ps[:, 0, toff:TJ], k_sb[:, hp, ssl],
                            q0[:, hp, toff:TJ], start=True, stop=True)
                        nc.tensor.matmul(
                            s_ps[:, 1, toff:TJ], k_sb[:, hp, ssl],
                            q1[:, hp, toff:TJ], start=True, stop=True)
                        state["pe"] += 2 * ((TJ - toff) * PE_NS + MM_OH)
                        p_sb = ppool.tile([128, 2, TJ], BF16, name="p_sb")
                        nc.scalar.activation(
                            p_sb[:, :, toff:TJ], s_ps[:, :, toff:TJ],
                            mybir.ActivationFunctionType.Exp, scale=SCALE)
                        state["act"] += (2 * (TJ - toff)) * ACT_NS + 170.0
                        if r >= 0:
                            # DVE (2x bf16 mode, ~250ns) keeps the AV's mask
                            # dependency off the Pool queue where 1us norm
                            # mults would delay it.
                            nc.vector.tensor_tensor(
                                out=p_sb[:, :, toff:toff + 128],
                                in0=p_sb[:, :, toff:toff + 128],
                                in1=tril_sb[:, None, :].to_broadcast(
                                    (128, 2, 128)),
                                op=mybir.AluOpType.mult)
                        # deadline (next block's Q/K/V) must precede the AV
                        # that may consume v_sb; pump (out-proj) after.
                        deadline_step((chunk_base + si + 3) / chunks_total)
                        if prev is not None:
                            emit_av(*prev)
                        prev = (p_sb, si, toff, nsi, o_ps0, o_ps1, hp)
                        pump()
                    emit_av(*prev)
                    pump()
                    return o_ps0, o_ps1

                def emit_av(p_sb, si, toff, nsi, o_ps0, o_ps1, hp):
                    for gg, o_ps in ((0, o_ps0), (1, o_ps1)):
                        nc.tensor.matmul(
                            o_ps[:, toff:TJ],
                            v_sb[si][:, 2 * hp + gg, :],
                            p_sb[:, gg, toff:TJ],
                            start=(si == 0), stop=(si == nsi - 1))
                    state["pe"] += 2 * ((TJ - toff) * PE_NS + MM_OH)

                def attn_norm(hp, tj, o_ps0, o_ps1):
                    # o_ps rows: 0..63 = O^T rows, 64 = L (ones column).
                    tsl = slice(tj * TJ, (tj + 1) * TJ)
                    u = hp * NTJ + tj
                    o_stage = npool.tile([DH + 1, 2, TJ], F32, name="o_stage",
                                         bufs=3)
                    nc.vector.tensor_copy(o_stage[:, 0, :], o_ps0)
                    nc.vector.tensor_copy(o_stage[:, 1, :], o_ps1)
                    nc.sync.dma_start(out=l_dram.ap()[u],
                                      in_=o_stage[DH:DH + 1, :, :])
                    lb = npool.tile([64, 2, TJ], F32, name="lb")
                    nc.sync.dma_start(
                        out=lb,
                        in_=l_dram.ap()[u:u + 1].to_broadcast((64, 2, TJ)))
                    linv = npool.tile([64, 2, TJ], F32, name="linv")
                    nc.vector.reciprocal_approx_fast(linv, lb)
                    nc.gpsimd.tensor_tensor(
                        out=o_sb[hp][0:64, tsl],
                        in0=o_stage[0:DH, 0, :],
                        in1=linv[:, 0, :], op=mybir.AluOpType.mult)
                    o_tmp = npool.tile([64, TJ], BF16, name="o_tmp")
                    nc.gpsimd.tensor_tensor(
                        out=o_tmp, in0=o_stage[0:DH, 1, :],
                        in1=linv[:, 1, :], op=mybir.AluOpType.mult)
                    nc.sync.dma_start(out=o_sb[hp][64:128, tsl], in_=o_tmp)

                import itertools
                for tj in range(NTJ):
                    gens = []
                    if tj == 0:
                        gens.append(v_stream(0))
                    if tj + 1 < NTJ:
                        gens.append(qk_stream(tj + 1))
                        gens.append(v_stream(tj + 1))
                    set_deadline(itertools.chain(*gens))
                    state["pe"] = state["act"] = 0.0
                    chunks_total = HP * (4 * tj + 4)
                    chunk_base = 0
                    for hp in range(HP):
                        o_ps0, o_ps1 = attn_unit(hp, tj, chunk_base,
                                                 chunks_total)
                        chunk_base += 4 * tj + 4
                        attn_norm(hp, tj, o_ps0, o_ps1)
                        if tj == NTJ - 1 and hp == HP - 2:
                            for ti in range(4 * tj, 4 * tj + 4):
                                pump_q.append(proj_partial_stream(ti))
                    deadline_step(1.0)
                    if tj == NTJ - 1:
                        drain_pump()
                        for ti in range(4 * tj, 4 * tj + 4):
                            proj_final(ti)
                    else:
                        for ti in range(4 * tj, 4 * tj + 4):
                            pump_q.append(proj_stream(ti))
                drain_pump()

    nc.compile()
    return nc


def _get_nc():
    if "nc" not in _NC_CACHE:
        _NC_CACHE["nc"] = _build()
    return _NC_CACHE["nc"]


def _pmajor8(a):
    """[C, N] -> [128, CC, N]: (p, c, n) = a[128c + p, n]."""
    n = a.shape[1]
    return np.ascontiguousarray(
        a.reshape(CC, 128, n).transpose(1, 0, 2)).astype(NP_BF16)


def _make_in_maps(x, Wq, Wk, Wv, Wp):
    tril01_h = np.where(
        np.arange(128)[:, None] > np.arange(128)[None, :],
        np.float32(0.0), np.float32(1.0)).astype(NP_BF16)
    in_maps = []
    for core in range(8):
        b, g = core // 2, core % 2
        heads = range(g * HG, (g + 1) * HG)
        wq = np.concatenate([Wq[h] for h in heads], axis=1)
        wk = np.concatenate([Wk[h] for h in heads], axis=1)
        wv_ = np.concatenate([Wv[h] for h in heads], axis=1)
        xt = np.ascontiguousarray(x[b].T)
        wp_ = Wp[g * HG * DH:(g + 1) * HG * DH, :]
        in_maps.append({
            "xin": _pmajor8(xt),
            "wqk": _pmajor8(np.concatenate([wq, wk], axis=1)),
            "wv": _pmajor8(wv_),
            "wpm": np.ascontiguousarray(
                wp_.reshape(HP, 128, C).transpose(1, 0, 2)).astype(NP_BF16),
            "tril01": tril01_h,
            "ones8": np.ones((128, HG), NP_BF16),
            "zrow": np.zeros((1, HP * TJ), NP_BF16),
        })
    return in_maps


_LAST_RESULTS = {}


def kernel(x, Wq, Wk, Wv, Wp, bp):
    x = np.asarray(x, np.float32)
    Wq = np.asarray(Wq, np.float32)
    Wk = np.asarray(Wk, np.float32)
    Wv = np.asarray(Wv, np.float32)
    Wp = np.asarray(Wp, np.float32)
    bp = np.asarray(bp, np.float32)

    nc = _get_nc()
    in_maps = _make_in_maps(x, Wq, Wk, Wv, Wp)
    res = bass_utils.run_bass_kernel_spmd(
        nc, in_maps, core_ids=list(range(8)), trace=TRACE)
    _LAST_RESULTS["res"] = res

    out = np.empty((B, T, C), np.float32)
    for b in range(B):
        out[b] = (res.results[2 * b]["out"].astype(np.float32)
                  + res.results[2 * b + 1]["out"].astype(np.float32) + bp)
    return out
